# revision 22
# baseline (speedup 1.0000x reference)
"""GQA attention kernel for Trainium2, 8-core batch x kv-head-group parallel.

Sharding: core (b, g2) = (core//4, core%4) handles batch b, kv heads
{2*g2, 2*g2+1}, q heads 8*g2..8*g2+7 (column-sharded q/k/v projections,
row-sharded o projection). Each core emits a bf16 partial [S, HIDDEN];
host sums 4 partials per batch.

Per-core dataflow (bf16 matmuls, fp32 PSUM):
  hidT (host-pre-transposed) [D, S] -> SBUF, plain DMA (no xbar transpose)
  Q:  wq-chunk stationary x hT -> qT pair [128(2h x 64d), 512tok]; RoPE via
      partition-offset DVE ops in d-major layout
  KV: packed (K|V) stationary -> [128, 512]; K RoPE d-major, stored
      duplicated on partitions 0:64/64:128 for row-tiled scores;
      V PE-transposed into [keys, 64|ones64] stationary blocks
  scores: 64x128 row-tiled matmul pair (2 q heads concurrent), causal
      trimming on diagonal chunks, exp on ScalarE over both heads at once
  AV: V|ones stationary -> av[0:64] + rowsum broadcast on rows 64:128 free;
      1/rowsum via reciprocal_approx_fast; o_proj row-parallel -> bf16 partial
Next-block projections are interleaved into the attention chunk loop to keep
the PE busy while ScalarE works through the exps.
"""
import numpy as np
import ml_dtypes

HIDDEN = 2048
N_HEADS = 32
N_KV = 8
HD = 64
B, S = 2, 2048
QB = 512            # q-block
NBLK = S // QB      # 4 q-blocks per core (one batch per core)
DCH = HIDDEN // 128  # 16 D-chunks
NPAIR = 4           # 4 q-head pairs per core (8 q heads)
ROPE_BASE = 500000.0


def build_nc(debug=False):
    import concourse.bass as bass
    import concourse.bacc as bacc
    import concourse.mybir as mybir
    import concourse.tile as tile

    F32 = mybir.dt.float32
    BF16 = mybir.dt.bfloat16
    F8 = mybir.dt.float8e4
    DR = mybir.MatmulPerfMode.DoubleRow
    EXP = mybir.ActivationFunctionType.Exp

    nc = bacc.Bacc()
    hidT = nc.dram_tensor("hidT", [128, NBLK, DCH, QB], BF16,
                          kind="ExternalInput")
    wq = nc.dram_tensor("wq", [128, 4, DCH, 128], BF16, kind="ExternalInput")
    wkv = nc.dram_tensor("wkv", [128, 2, DCH, 128], BF16, kind="ExternalInput")
    wo = nc.dram_tensor("wo", [128, NPAIR, HIDDEN], BF16, kind="ExternalInput")
    cosd = nc.dram_tensor("cosd", [128, S], BF16, kind="ExternalInput")
    sind = nc.dram_tensor("sind", [128, S], BF16, kind="ExternalInput")
    outp = nc.dram_tensor("outp", [S, HIDDEN], BF16, kind="ExternalOutput")
    if debug:
        dbg_qt = nc.dram_tensor("dbg_qt", [128, QB], BF16, kind="ExternalOutput")
        dbg_kt = nc.dram_tensor("dbg_kt", [128, S], BF16, kind="ExternalOutput")
        dbg_v = nc.dram_tensor("dbg_v", [128, S // 128, 128], BF16,
                               kind="ExternalOutput")
        dbg_esb = nc.dram_tensor("dbg_esb", [128, 2, QB], BF16,
                                 kind="ExternalOutput")
        dbg_av = nc.dram_tensor("dbg_av", [128, 2, QB], F32,
                                kind="ExternalOutput")
        dbg_avt = nc.dram_tensor("dbg_avt", [128, QB], BF16,
                                 kind="ExternalOutput")

    with tile.TileContext(nc) as tc:
        with (
            tc.tile_pool(name="singles", bufs=1) as sg,
            tc.tile_pool(name="work", bufs=2) as wk_p,
            tc.tile_pool(name="qtp", bufs=8) as qtp,
            tc.tile_pool(name="avtp", bufs=2) as avtp,
            tc.tile_pool(name="expp", bufs=4) as ep,
            tc.tile_pool(name="psq", bufs=1, space="PSUM") as pq,
            tc.tile_pool(name="pss", bufs=2, space="PSUM") as pss,
            tc.tile_pool(name="psav", bufs=3, space="PSUM") as pav,
        ):
            # ---- resident weights / tables / caches ----
            # all inputs are host-pre-arranged into SBUF layout, so every
            # load is 128 contiguous per-partition runs; order favors the
            # critical path (wq pair 0 + block 0 hidden first).
            wq_s = sg.tile([128, 4, DCH, 128], BF16)
            nc.sync.dma_start(out=wq_s[:, 0], in_=wq[:, 0])
            hid_s = sg.tile([128, NBLK, DCH, QB], BF16)
            for j2 in range(8):
                nc.sync.dma_start(out=hid_s[:, 0, j2 * 2:(j2 + 1) * 2],
                                  in_=hidT[:, 0, j2 * 2:(j2 + 1) * 2])
            cos_s = sg.tile([128, S], BF16)
            nc.sync.dma_start(out=cos_s, in_=cosd[:, :])
            sin_s = sg.tile([128, S], BF16)
            nc.sync.dma_start(out=sin_s, in_=sind[:, :])
            wkv_s = sg.tile([128, 2, DCH, 128], BF16)
            nc.sync.dma_start(out=wkv_s, in_=wkv[:, :, :, :])
            for cc in range(1, 4):
                nc.sync.dma_start(out=wq_s[:, cc], in_=wq[:, cc])
            for qb in range(1, NBLK):
                nc.sync.dma_start(out=hid_s[:, qb], in_=hidT[:, qb])
            wo_s = sg.tile([128, NPAIR, HIDDEN], BF16)
            for p in range(NPAIR):
                nc.sync.dma_start(out=wo_s[:, p], in_=wo[:, p])

            ident = sg.tile([128, 128], BF16)
            nc.vector.memset(ident, 1.0)
            nc.gpsimd.affine_select(
                out=ident, in_=ident, compare_op=mybir.AluOpType.is_equal,
                fill=0.0, base=0, pattern=[[-1, 128]], channel_multiplier=1)

            # K cache, duplicated on partition halves for row-tiled scores
            KT = [sg.tile([128, S], BF16, name=f"KT{g}") for g in range(2)]
            # V cache: [keys(128), chunk, 64 V | 64 ones]
            Vst = [sg.tile([128, S // 128, 128], BF16, name=f"Vst{g}")
                   for g in range(2)]
            for g in range(2):
                nc.vector.memset(Vst[g][:, :, 64:128], 1.0)

            qT = [[None] * NPAIR for _ in range(NBLK)]   # roped q, [128(2h),512]
            AVT = [None] * NBLK  # normalized av^T, fp8 [128, pair, 512]

            def qproj_steps(qb, p):
                """Q projection chunk for pair p + RoPE -> qT[qb][p]."""
                tcols = slice(qb * QB, (qb + 1) * QB)
                psq = pq.tile([128, QB], F32, tag="mm")
                for j in range(DCH):
                    yield lambda j=j, psq=psq: nc.tensor.matmul(
                        psq, wq_s[:, p, j, :],
                        hid_s[:, qb, j, :],
                        start=(j == 0), stop=(j == DCH - 1))

                def rope():
                    q_sb = wk_p.tile([128, QB], BF16, tag="qsb")
                    nc.vector.tensor_copy(out=q_sb, in_=psq)
                    t2 = wk_p.tile([128, QB], BF16, tag="t2")
                    for h2 in range(2):
                        b0 = h2 * 64
                        nc.vector.tensor_mul(out=t2[b0:b0 + 32],
                                             in0=q_sb[b0 + 32:b0 + 64],
                                             in1=sin_s[b0 + 32:b0 + 64, tcols])
                        nc.vector.tensor_mul(out=t2[b0 + 32:b0 + 64],
                                             in0=q_sb[b0:b0 + 32],
                                             in1=sin_s[b0:b0 + 32, tcols])
                    t1 = wk_p.tile([128, QB], BF16, tag="t1")
                    nc.vector.tensor_mul(out=t1, in0=q_sb, in1=cos_s[:, tcols])
                    qt = qtp.tile([128, QB], BF16, tag="qT")
                    nc.vector.tensor_add(out=qt, in0=t1, in1=t2)
                    qT[qb][p] = qt
                    if debug and qb == 0 and p == 0:
                        nc.sync.dma_start(out=dbg_qt[:, :], in_=qt)
                yield rope

            def kvproj_steps(qb, g):
                """K|V projection for kv head g + K RoPE + V transpose."""
                tcols = slice(qb * QB, (qb + 1) * QB)
                pkv = pq.tile([128, QB], F32, tag="mm")
                for j in range(DCH):
                    yield lambda j=j, pkv=pkv: nc.tensor.matmul(
                        pkv, wkv_s[:, g, j, :],
                        hid_s[:, qb, j, :],
                        start=(j == 0), stop=(j == DCH - 1))

                def k_rope():
                    k_sb = wk_p.tile([64, QB], BF16, tag="ksb")
                    nc.vector.tensor_copy(out=k_sb, in_=pkv[0:64, :])
                    kt2 = wk_p.tile([64, QB], BF16, tag="kt2")
                    nc.vector.tensor_mul(out=kt2[0:32], in0=k_sb[32:64],
                                         in1=sin_s[32:64, tcols])
                    nc.vector.tensor_mul(out=kt2[32:64], in0=k_sb[0:32],
                                         in1=sin_s[0:32, tcols])
                    kt1 = wk_p.tile([64, QB], BF16, tag="kt1")
                    nc.vector.tensor_mul(out=kt1, in0=k_sb, in1=cos_s[0:64, tcols])
                    nc.vector.tensor_add(out=KT[g][0:64, tcols], in0=kt1, in1=kt2)
                    nc.gpsimd.tensor_copy(out=KT[g][64:128, tcols],
                                          in_=KT[g][0:64, tcols])
                yield k_rope

                v_sb = wk_p.tile([64, QB], BF16, tag="vsb")
                yield lambda: nc.vector.tensor_copy(out=v_sb, in_=pkv[64:128, :])
                for a in range(QB // 128):
                    def v_tr(a=a, v_sb=v_sb):
                        ptv = pq.tile([128, 64], BF16, tag="mm")
                        nc.tensor.matmul(ptv, v_sb[:, a * 128:(a + 1) * 128],
                                         ident[0:64, 0:64],
                                         is_transpose=True, start=True, stop=True)
                        nc.vector.tensor_copy(out=Vst[g][:, qb * 4 + a, 0:64],
                                              in_=ptv)
                    yield v_tr

            def proj_block_steps(qb):
                for p in range(NPAIR):
                    yield from qproj_steps(qb, p)
                    if p < 2:
                        yield from kvproj_steps(qb, p)

            def emit_attention(qb, fillers, boundary_fillers):
                """Attention for all 4 pairs of block qb; pulls filler steps
                (next block's projections) into the chunk loop."""
                nkc = 4 * (qb + 1)
                for p in range(NPAIR):
                    g = p // 2
                    qt = qT[qb][p]
                    avh = [pav.tile([128, QB], F32, tag="av", name=f"av{h2}")
                           for h2 in range(2)]
                    esbs = [None] * nkc

                    def scores(kc):
                        trim = max(0, kc - 4 * qb) * 128
                        pS2 = pss.tile([128, 2, QB], F32, tag="s2")
                        for h2 in range(2):
                            nc.tensor.matmul(
                                pS2[:, h2, trim:],
                                KT[g][h2 * 64:(h2 + 1) * 64,
                                      kc * 128:(kc + 1) * 128],
                                qt[h2 * 64:(h2 + 1) * 64, trim:],
                                start=True, stop=True)
                        esb = ep.tile([128, 2, QB], BF16, tag="esb")
                        nc.scalar.activation(esb[:, :, trim:], pS2[:, :, trim:],
                                             EXP, scale=0.125)
                        if trim > 0 or kc == 4 * qb:
                            for h2 in range(2):
                                nc.gpsimd.affine_select(
                                    out=esb[:, h2, trim:trim + 128],
                                    in_=esb[:, h2, trim:trim + 128],
                                    compare_op=mybir.AluOpType.is_ge, fill=0.0,
                                    base=0, pattern=[[1, 128]],
                                    channel_multiplier=-1)
                        esbs[kc] = esb
                        if debug and qb == 0 and p == 0 and kc == 0:
                            nc.sync.dma_start(out=dbg_esb[:, :, :], in_=esb)

                    def av(kc):
                        trim = max(0, kc - 4 * qb) * 128
                        for h2 in range(2):
                            nc.tensor.matmul(
                                avh[h2][:, trim:], Vst[g][:, kc, :],
                                esbs[kc][:, h2, trim:],
                                start=(kc == 0), stop=(kc == nkc - 1))

                    scores(0)
                    if nkc > 1:
                        scores(1)
                    for kc in range(2, nkc):
                        for _ in range(2):
                            f = next(fillers, None)
                            if f is not None:
                                f()
                        av(kc - 2)
                        scores(kc)
                    if nkc > 1:
                        av(nkc - 2)
                    av(nkc - 1)

                    if debug and qb == 0 and p == 0:
                        for h2 in range(2):
                            avsb = wk_p.tile([128, QB], F32, tag="avsb",
                                             name=f"avsb{h2}")
                            nc.vector.tensor_copy(out=avsb, in_=avh[h2])
                            nc.sync.dma_start(out=dbg_av[:, h2, :], in_=avsb)
                    # normalize: rows 64:128 of av hold rowsum broadcast
                    if p == 0:
                        AVT[qb] = avtp.tile([128, NPAIR, QB], BF16, tag="avt",
                                            name=f"avt{qb}")
                    for h2 in range(2):
                        # NB: reciprocal_approx_fast mis-reads PSUM sources;
                        # bounce the rowsum through SBUF first.
                        rs_sb = wk_p.tile([64, QB], F32, tag="rs_sb")
                        nc.scalar.copy(out=rs_sb, in_=avh[h2][64:128, :])
                        rrb = wk_p.tile([64, QB], F32, tag="rrb")
                        nc.vector.reciprocal_approx_fast(out=rrb, in_=rs_sb)
                        nc.vector.tensor_mul(
                            out=AVT[qb][h2 * 64:(h2 + 1) * 64, p, :],
                            in0=avh[h2][0:64, :], in1=rrb)
                    f = next(boundary_fillers, None)
                    if f is not None:
                        f()

            def oproj_steps(qb):
                for a in range(QB // 128):
                    osb = wk_p.tile([128, HIDDEN], BF16, tag="osb",
                                    name=f"osb{qb}_{a}")

                    def ostep(a=a, osb=osb):
                        r0 = qb * QB + a * 128
                        for half in range(2):
                            po = pss.tile([128, 2, QB], F32, tag="s2",
                                          name=f"po{half}")
                            for p in range(NPAIR):
                                for n2 in range(2):
                                    nch = half * 2 + n2
                                    nc.tensor.matmul(
                                        po[:, n2, :],
                                        AVT[qb][:, p, a * 128:(a + 1) * 128],
                                        wo_s[:, p, nch * QB:(nch + 1) * QB],
                                        start=(p == 0), stop=(p == NPAIR - 1))
                            if half == 0:
                                nc.vector.tensor_copy(out=osb[:, 0:1024],
                                                      in_=po)
                            else:
                                nc.scalar.copy(out=osb[:, 1024:2048], in_=po)
                            nc.sync.dma_start(
                                out=outp[r0:r0 + 128,
                                         half * 1024:(half + 1) * 1024],
                                in_=osb[:, half * 1024:(half + 1) * 1024])
                    yield ostep

            # ---- main schedule ----
            for f in proj_block_steps(0):
                f()
            empty = iter(())
            prev_oproj = empty
            for qb in range(NBLK):
                fillers = proj_block_steps(qb + 1) if qb + 1 < NBLK else empty
                emit_attention(qb, fillers, prev_oproj)
                for f in fillers:  # leftover projection work
                    f()
                for f in prev_oproj:  # leftover o_proj of previous block
                    f()
                prev_oproj = oproj_steps(qb)
            for f in prev_oproj:
                f()
            if debug:
                nc.sync.dma_start(out=dbg_kt[:, :], in_=KT[0])
                nc.sync.dma_start(out=dbg_v[:, :, :], in_=Vst[0])
    nc.compile()
    return nc


_NC_CACHE = {}


def _get_nc():
    if "nc" not in _NC_CACHE:
        _NC_CACHE["nc"] = build_nc()
    return _NC_CACHE["nc"]


def _rope_tables():
    """d-major cos / signed-sin tables, [128, S] = 2 heads x 64 d rows."""
    inv = 1.0 / (ROPE_BASE ** (np.arange(0, HD, 2, dtype=np.float64) / HD))  # 32
    t = np.arange(S, dtype=np.float64)
    fr = np.outer(inv, t)                        # [32, S]
    cos1 = np.concatenate([np.cos(fr), np.cos(fr)], 0)    # [64, S]
    # sin rows are indexed by the INPUT partition of the rotate-mul:
    # q[0:32]*sin -> out[32:64] (+), q[32:64]*sin -> out[0:32] (-)
    sgn1 = np.concatenate([np.sin(fr), -np.sin(fr)], 0)   # [64, S]
    bf = ml_dtypes.bfloat16
    cosd = np.tile(cos1, (2, 1)).astype(bf)      # [128, S]
    sind = np.tile(sgn1, (2, 1)).astype(bf)
    return cosd, sind


def _prepare_in_maps(hidden_states, w_q, w_k, w_v, w_o):
    bf = ml_dtypes.bfloat16
    def dproj(w):  # [2048, C*128] -> [128p, C, DCH, 128] (SBUF layout)
        c = w.shape[1] // 128
        return np.ascontiguousarray(
            w.reshape(DCH, 128, c, 128).transpose(1, 2, 0, 3)).astype(bf)

    h = np.asarray(hidden_states).reshape(B, S, HIDDEN)
    hT = []
    for b in range(B):
        a = h[b].T.reshape(DCH, 128, NBLK, QB).transpose(1, 2, 0, 3)
        hT.append(np.ascontiguousarray(a).astype(bf))
    cosd, sind = _rope_tables()
    in_maps = []
    for core in range(8):
        b, g2 = core // 4, core % 4
        a = 128 * g2
        wkv_c = np.concatenate(
            [w_k[:, a:a + 64], w_v[:, a:a + 64],
             w_k[:, a + 64:a + 128], w_v[:, a + 64:a + 128]], axis=1)
        wo_c = w_o[512 * g2:512 * (g2 + 1), :]
        in_maps.append({
            "hidT": hT[b],
            "wq": dproj(w_q[:, 512 * g2:512 * (g2 + 1)]),
            "wkv": dproj(wkv_c),
            "wo": np.ascontiguousarray(
                wo_c.reshape(NPAIR, 128, HIDDEN).transpose(1, 0, 2)).astype(bf),
            "cosd": cosd,
            "sind": sind,
        })
    return in_maps


def kernel(hidden_states, attention_mask, w_q, w_k, w_v, w_o):
    from concourse.bass_utils import run_bass_kernel_spmd

    in_maps = _prepare_in_maps(hidden_states, w_q, w_k, w_v, w_o)
    nc = _get_nc()
    res = run_bass_kernel_spmd(nc, in_maps, list(range(8)))
    out = np.zeros((B, S, HIDDEN), dtype=np.float32)
    for core, r in enumerate(res.results):
        out[core // 4] += r["outp"].astype(np.float32)
    return out
